# revision 2
# baseline (speedup 1.0000x reference)
"""Trainium2 Bass kernel for nn_DuelingDQN (moe_routing).

Strategy (hardware time is all that counts; host prep is free):
  * Pure data parallel over 8 cores; batch rows are routed (sorted) by
    event_type on the host so each 512-sample supertile uses exactly one
    advantage head -> no on-device select, head weights picked statically.
  * Feature-major layout on device: activations live as [features, batch]
    so every layer is a weights-stationary PE matmul with N=512 moving
    columns and zero transposes. The input is transposed on the host.
  * LayerNorm mean is folded into pre-centered weights (host, f64).
    The per-sample 1/std is NOT applied on device; it is deferred through
    the relu (r>0) and the next matmul, with each layer's bias injected as
    a rank-1 K=1 matmul  (bias_row  x  running_std_row).  Exact algebra:
      u_l   = relu(gamma_l * v_l)                    (device tensor)
      v_l   = Wc_l^T u_{l-1} + bc_l * srow_{l-1}     (PE, PSUM)
      s2_l  = mean(v_l^2) + eps * s2_{l-1}           (stats)
      srow_l= sqrt(s2_l);  true h_l = u_l / srow_l
  * Per-sample stats: ones-vector matmul on PE -> [1,512] PSUM row ->
    one-partition copy to SBUF -> tiny SBUF->SBUF DMA "transpose" into a
    [128,4] tile where sqrt/reciprocal/eps-chain run on all lanes.
  * Final dueling combine: head-2 matmul is run with the activations as
    the stationary operand, which yields the output directly batch-major
    [128 samples, 33]; value/advantage de-deferral + bias + mean-free
    advantage (Wa2 pre-centered over actions) are 4 small DVE ops, then a
    single contiguous DMA writes [512,32] rows to DRAM.
"""

import os
import sys
from contextlib import ExitStack

os.environ.setdefault("MYCRO_LOCAL_CACHE", "1")
if "/opt/trn_rl_repo" not in sys.path:
    sys.path.insert(0, "/opt/trn_rl_repo")

import numpy as np

NCORES = 8
TILE = 512          # samples per supertile (PE moving-operand max for fp32)
SUB = 128           # samples per partition-subtile
G = TILE // SUB     # 4
EPS = 1e-5
S_DIM = 199
D_IN = S_DIM + 1    # 200 (state + time feature)
A = 32
E = 3

LAST_EXEC_NS = None
_PROG_CACHE = {}


def _build_program(R, tile_events):
    import concourse.bass as bass
    import concourse.tile as tile
    from concourse import bacc, mybir

    f32 = mybir.dt.float32
    f32r = mybir.dt.float32r   # TF32-ish PE input: 4x matmul throughput vs fp32
    _no = os.environ.get
    dt_main = f32 if _no("NO_F32R_MAIN") else f32r
    dt_stats = f32 if _no("NO_F32R_STATS") else f32r
    dt_bias = f32 if _no("NO_F32R_BIAS") else f32r
    dt_qt = f32 if _no("NO_F32R_QT") else f32r
    AF = mybir.ActivationFunctionType
    OP = mybir.AluOpType

    nc = bacc.Bacc("TRN2", target_bir_lowering=False, debug=False,
                   enable_asserts=True, num_devices=NCORES)

    def din(name, shape, dt=f32):
        return nc.dram_tensor(name, list(shape), dt, kind="ExternalInput").ap()

    xT_d = din("xT", [D_IN + 1, R], dt_main)     # rows: 199 state + td + ones
    w1k0_d = din("w1k0", [128, 256], dt_main)
    w1k1_d = din("w1k1", [73, 256], dt_main)
    w2k0_d = din("w2k0", [128, 256], dt_main)
    w2k1_d = din("w2k1", [128, 256], dt_main)
    w3k0_d = din("w3k0", [128, 128], dt_main)
    w3k1_d = din("w3k1", [128, 128], dt_main)
    b2_d = din("b2row", [1, 256], dt_bias)
    b3_d = din("b3row", [1, 128], dt_bias)
    wh1_d = din("wh1", [E, 128, 128], dt_main)
    bh1_d = din("bh1", [E, 1, 128], dt_bias)
    wh2_d = din("wh2", [E, 128, 34], dt_qt)
    ba2_d = din("ba2t", [E, 128, A])
    g1_d = din("g1t", [128, 2])
    g2_d = din("g2t", [128, 2])
    g3_d = din("g3t", [128, 1])
    gh_d = din("ght", [E, 128, 1])
    on256_d = din("ones256", [128, 2], dt_stats)
    on128_d = din("ones128", [128, 2], dt_stats)
    gones_d = din("gones", [128, 33], dt_stats)
    epsc_d = din("epsc", [128, 1])
    out_d = nc.dram_tensor("out", [R, A], f32, kind="ExternalOutput").ap()
    scr_d = nc.dram_tensor("scr", [2 * len(tile_events), TILE], f32).ap()

    with tile.TileContext(nc) as tc, ExitStack() as ctx:
        PS = bass.MemorySpace.PSUM
        def _b(name, d):
            return int(os.environ.get(f"BUFS_{name}", d))
        wp = ctx.enter_context(tc.tile_pool(name="w", bufs=1))
        xp = ctx.enter_context(tc.tile_pool(name="x", bufs=_b("X", 6)))
        sqp = ctx.enter_context(tc.tile_pool(name="sq", bufs=_b("SQ", 8)))
        up = ctx.enter_context(tc.tile_pool(name="u", bufs=_b("U", 8)))
        rp = ctx.enter_context(tc.tile_pool(name="r", bufs=_b("R", 8)))
        tp = ctx.enter_context(tc.tile_pool(name="t", bufs=_b("T", 10)))
        op_ = ctx.enter_context(tc.tile_pool(name="o", bufs=_b("O", 8)))
        zp = ctx.enter_context(tc.tile_pool(name="z", bufs=_b("Z", 6), space=PS))
        sp = ctx.enter_context(tc.tile_pool(name="s", bufs=_b("S", 2), space=PS))

        def wtile(d_ap, shape, tag, dt=f32):
            t = wp.tile(list(shape), dt, tag=tag, name=tag)
            nc.sync.dma_start(t[:], d_ap)
            return t

        w1k0 = wtile(w1k0_d, [128, 256], "w1k0", dt_main)
        w1k1 = wtile(w1k1_d, [73, 256], "w1k1", dt_main)
        w2k0 = wtile(w2k0_d, [128, 256], "w2k0", dt_main)
        w2k1 = wtile(w2k1_d, [128, 256], "w2k1", dt_main)
        w3k0 = wtile(w3k0_d, [128, 128], "w3k0", dt_main)
        w3k1 = wtile(w3k1_d, [128, 128], "w3k1", dt_main)
        b2r = wtile(b2_d, [1, 256], "b2r", dt_bias)
        b3r = wtile(b3_d, [1, 128], "b3r", dt_bias)
        wh1 = [wtile(wh1_d[e], [128, 128], f"wh1_{e}", dt_main) for e in range(E)]
        bh1 = [wtile(bh1_d[e], [1, 128], f"bh1_{e}", dt_bias) for e in range(E)]
        wh2 = [wtile(wh2_d[e], [128, 34], f"wh2_{e}", dt_qt) for e in range(E)]
        ba2 = [wtile(ba2_d[e], [128, A], f"ba2_{e}") for e in range(E)]
        g1t = wtile(g1_d, [128, 2], "g1t")
        g2t = wtile(g2_d, [128, 2], "g2t")
        g3t = wtile(g3_d, [128, 1], "g3t")
        ght = [wtile(gh_d[e], [128, 1], f"ght_{e}") for e in range(E)]
        on256 = wtile(on256_d, [128, 2], "on256", dt_stats)
        on128 = wtile(on128_d, [128, 2], "on128", dt_stats)
        gones = wtile(gones_d, [128, 33], "gones", dt_stats)
        epsc = wtile(epsc_d, [128, 1], "epsc")

        def mm(out, lhsT, rhs, start, stop):
            nc.tensor.matmul(out, lhsT, rhs, start=start, stop=stop)

        def relu_dve(dst, src, gamma):
            nc.vector.tensor_scalar(dst, src, gamma, 0.0, OP.mult, OP.max)

        def relu_act(dst, src, gamma):
            nc.scalar.activation(dst, src, AF.Relu, scale=gamma)

        gamma_ones = os.environ.get("KGAMMA1", "1") == "1"
        _dma_eng = os.environ.get("DMA_SMALL", "sync")
        DMA_SMALL = getattr(nc, _dma_eng).dma_start

        def make_stages(t_i, ev):
            """Stage closures for one supertile; emitted interleaved across
            tiles so in-order engine queues always hold independent work."""
            c0 = t_i * TILE
            cols = slice(c0, c0 + TILE)
            v = {}

            def s_load():
                v["x0"] = xp.tile([128, TILE], dt_main, tag="x0", name="x0")
                nc.sync.dma_start(v["x0"][:], xT_d[0:128, cols])
                v["x1"] = xp.tile([73, TILE], dt_main, tag="x1", name="x1")
                nc.sync.dma_start(v["x1"][:], xT_d[128:201, cols])

            def s_l1():
                z1a = zp.tile([128, TILE], f32, tag="z", name="z1a")
                mm(z1a[:], w1k0[:, 0:128], v["x0"][:], True, False)
                mm(z1a[:], w1k1[:, 0:128], v["x1"][:], False, True)
                z1b = zp.tile([128, TILE], f32, tag="z", name="z1b")
                mm(z1b[:], w1k0[:, 128:256], v["x0"][:], True, False)
                mm(z1b[:], w1k1[:, 128:256], v["x1"][:], False, True)
                v["z1a"], v["z1b"] = z1a, z1b

            def s_l1p():
                z1a, z1b = v["z1a"], v["z1b"]
                stats1 = sp.tile([2, TILE], f32, tag="stats", name="stats1")
                sq1a = sqp.tile([128, TILE], dt_stats, tag="sq", name="sq1a")
                nc.scalar.square(sq1a[:], z1a[:])
                sq1b = sqp.tile([128, TILE], dt_stats, tag="sq", name="sq1b")
                nc.scalar.square(sq1b[:], z1b[:])
                mm(stats1[0:2, :], on256[:], sq1a[:], True, False)
                mm(stats1[0:2, :], on256[:], sq1b[:], False, True)
                u1a = up.tile([128, TILE], dt_main, tag="u", name="u1a")
                relu_dve(u1a[:], z1a[:], g1t[:, 0:1])
                u1b = up.tile([128, TILE], dt_main, tag="u", name="u1b")
                relu_dve(u1b[:], z1b[:], g1t[:, 1:2])
                s1row = rp.tile([1, TILE], dt_bias, tag="rowb", name="s1row")
                nc.scalar.activation(s1row[:], stats1[0:1, :], AF.Sqrt,
                                     bias=epsc[0:1, :])
                v.update(u1a=u1a, u1b=u1b, s1row=s1row)

            def s_l2():
                z2a = zp.tile([128, TILE], f32, tag="z", name="z2a")
                mm(z2a[:], w2k0[:, 0:128], v["u1a"][:], True, False)
                mm(z2a[:], w2k1[:, 0:128], v["u1b"][:], False, False)
                mm(z2a[:], b2r[0:1, 0:128], v["s1row"][:], False, True)
                z2b = zp.tile([128, TILE], f32, tag="z", name="z2b")
                mm(z2b[:], w2k0[:, 128:256], v["u1a"][:], True, False)
                mm(z2b[:], w2k1[:, 128:256], v["u1b"][:], False, False)
                mm(z2b[:], b2r[0:1, 128:256], v["s1row"][:], False, True)
                v["z2a"], v["z2b"] = z2a, z2b

            def s_l2p():
                z2a, z2b = v["z2a"], v["z2b"]
                stats2 = sp.tile([2, TILE], f32, tag="stats", name="stats2")
                sq2a = sqp.tile([128, TILE], dt_stats, tag="sq", name="sq2a")
                nc.scalar.square(sq2a[:], z2a[:])
                sq2b = sqp.tile([128, TILE], dt_stats, tag="sq", name="sq2b")
                nc.scalar.square(sq2b[:], z2b[:])
                mm(stats2[0:2, :], on256[:], sq2a[:], True, False)
                mm(stats2[0:2, :], on256[:], sq2b[:], False, True)
                u2a = up.tile([128, TILE], dt_main, tag="u", name="u2a")
                relu_dve(u2a[:], z2a[:], g2t[:, 0:1])
                u2b = up.tile([128, TILE], dt_main, tag="u", name="u2b")
                relu_dve(u2b[:], z2b[:], g2t[:, 1:2])
                s2row = rp.tile([1, TILE], dt_bias, tag="rowb", name="s2row")
                nc.scalar.activation(s2row[:], stats2[0:1, :], AF.Sqrt,
                                     bias=epsc[0:1, :])
                v.update(u2a=u2a, u2b=u2b, s2row=s2row)
            def s_l3():
                z3 = zp.tile([128, TILE], f32, tag="z", name="z3")
                mm(z3[:], w3k0[:], v["u2a"][:], True, False)
                mm(z3[:], w3k1[:], v["u2b"][:], False, False)
                mm(z3[:], b3r[:], v["s2row"][:], False, True)
                v["z3"] = z3

            def s_l3p():
                z3 = v["z3"]
                stats3 = sp.tile([2, TILE], f32, tag="stats", name="stats3")
                sq3 = sqp.tile([128, TILE], dt_stats, tag="sq", name="sq3")
                nc.scalar.square(sq3[:], z3[:])
                mm(stats3[0:2, :], on128[:], sq3[:], True, True)
                u3 = up.tile([128, TILE], dt_main, tag="u", name="u3")
                relu_dve(u3[:], z3[:], g3t[:, 0:1])
                s3row = rp.tile([1, TILE], dt_bias, tag="rowb", name="s3row")
                nc.scalar.activation(s3row[:], stats3[0:1, :], AF.Sqrt,
                                     bias=epsc[0:1, :])
                v.update(u3=u3, s3row=s3row)

            def s_h():
                h = zp.tile([128, TILE], f32, tag="z", name="h")
                mm(h[:], wh1[ev][:], v["u3"][:], True, False)
                mm(h[:], bh1[ev][:], v["s3row"][:], False, True)
                v["h"] = h

            def s_hp():
                h = v["h"]
                statsh = sp.tile([33, TILE], f32, tag="stats", name="statsh")
                sqh = sqp.tile([128, TILE], dt_stats, tag="sq", name="sqh")
                nc.scalar.square(sqh[:], h[:])
                mm(statsh[0:33, :], gones[:], sqh[:], True, True)
                uh = up.tile([128, TILE], dt_qt, tag="uh", name="uh")
                relu_dve(uh[:], h[:], ght[ev][:, 0:1])
                svarow = rp.tile([33, TILE], f32, tag="rowb", name="svarow")
                nc.scalar.activation(svarow[:], statsh[0:33, :], AF.Sqrt,
                                     bias=epsc[0:33, :])
                rv, ra = 2 * t_i, 2 * t_i + 1
                DMA_SMALL(scr_d[rv:ra + 1, :], svarow[0:33:32, :])
                sT = tp.tile([128, 2, G], f32, tag="tt", name="sT")
                DMA_SMALL(
                    sT[:], scr_d[rv:ra + 1, :].rearrange("j (g p) -> p j g", p=SUB))
                cT = tp.tile([128, 2, G], f32, tag="tt", name="cT")
                nc.vector.reciprocal(cT[:], sT[:])
                v.update(uh=uh, cT=cT)

            def s_qt():
                uh = v["uh"]
                qT = zp.tile([128, G, 34], f32, tag="z", name="qT")
                for g in range(G):
                    mm(qT[:, g, :], uh[:, g * SUB:(g + 1) * SUB], wh2[ev][:],
                       True, True)
                v["qT"] = qT

            def s_fin():
                qT, cT = v["qT"], v["cT"]
                cvT = cT[:, 0, :]
                caT = cT[:, 1, :]
                valT = tp.tile([128, G], f32, tag="tt", name="valT")
                nc.vector.tensor_tensor(valT[:], qT[:, :, 0], cvT, OP.mult)
                badd = op_.tile([128, G, A], f32, tag="badd", name="badd")
                nc.vector.tensor_tensor(
                    badd[:],
                    ba2[ev][:, :].unsqueeze(1).broadcast_to([128, G, A]),
                    valT[:, :].unsqueeze(-1).broadcast_to([128, G, A]),
                    OP.add,
                )
                advs = op_.tile([128, G, A], f32, tag="advs", name="advs")
                nc.vector.tensor_tensor(
                    advs[:], qT[:, :, 1:33],
                    caT.unsqueeze(-1).broadcast_to([128, G, A]), OP.mult,
                )
                outt = op_.tile([128, G, A], f32, tag="outt", name="outt")
                nc.vector.tensor_tensor(outt[:], advs[:], badd[:], OP.add)
                DMA_SMALL(
                    out_d[cols, :].rearrange("(g p) a -> p g a", p=SUB), outt[:]
                )

            return [s_load, s_l1, s_l1p, s_l2, s_l2p, s_l3, s_l3p,
                    s_h, s_hp, s_qt, s_fin]

        SKEW = int(os.environ.get("SKEW", "3"))
        T_n = len(tile_events)
        all_stages = [make_stages(t, ev) for t, ev in enumerate(tile_events)]
        n_st = len(all_stages[0])
        # software-pipelined emission: tile t's stage s runs alongside tile
        # t+1's stage s-1, ... (skew window of SKEW tiles)
        for wave in range(T_n + (SKEW - 1)):
            # emit: at wave w, tile w runs early stages, tile w-1 late
            # stages; FINE=1 alternates single stages so every engine queue
            # interleaves the two tiles' work at op granularity
            if os.environ.get("FINE", "1") == "1":
                step = (n_st + SKEW - 1) // SKEW
                for i in range(step):
                    for lag in range(SKEW):
                        t = wave - lag
                        j = lag * step + i
                        if 0 <= t < T_n and j < n_st:
                            all_stages[t][j]()
            else:
                for lag in range(SKEW):
                    t = wave - lag
                    if not (0 <= t < T_n):
                        continue
                    s_lo = lag * n_st // SKEW
                    s_hi = (lag + 1) * n_st // SKEW
                    for s in range(s_lo, s_hi):
                        all_stages[t][s]()

    nc.compile()
    return nc


def _prep_weights(inp):
    """Center LN means into the weights (f64), build device weight arrays."""
    f8 = np.float64
    W1 = np.asarray(inp["W1"], f8); b1 = np.asarray(inp["b1"], f8)
    W2 = np.asarray(inp["W2"], f8); b2 = np.asarray(inp["b2"], f8)
    W3 = np.asarray(inp["W3"], f8); b3 = np.asarray(inp["b3"], f8)
    Wv1 = np.asarray(inp["Wv1"], f8); bv1 = np.asarray(inp["bv1"], f8)
    Wv2 = np.asarray(inp["Wv2"], f8); bv2 = np.asarray(inp["bv2"], f8)
    Wa1 = np.asarray(inp["Wa1"], f8); ba1 = np.asarray(inp["ba1"], f8)
    Wa2 = np.asarray(inp["Wa2"], f8); ba2 = np.asarray(inp["ba2"], f8)

    for k in ("be1", "be2", "be3", "bev", "bea"):
        if not np.allclose(np.asarray(inp[k]), 0.0):
            raise NotImplementedError(f"nonzero LN beta {k} unsupported")

    W1a = np.empty((201, 256), f8)
    W1a[:200] = W1
    W1a[200] = b1
    W1c = (W1a - W1a.mean(axis=1, keepdims=True)).astype(np.float32)
    W2c = (W2 - W2.mean(axis=1, keepdims=True)).astype(np.float32)
    b2c = (b2 - b2.mean()).astype(np.float32)
    W3c = (W3 - W3.mean(axis=1, keepdims=True)).astype(np.float32)
    b3c = (b3 - b3.mean()).astype(np.float32)

    wh1 = np.empty((E, 128, 128), np.float32)
    bh1 = np.empty((E, 1, 128), np.float32)
    wh2 = np.zeros((E, 128, 34), np.float32)
    ba2t = np.empty((E, 128, A), np.float32)
    for e in range(E):
        hv = Wv1 - Wv1.mean(axis=1, keepdims=True)
        ha = Wa1[e] - Wa1[e].mean(axis=1, keepdims=True)
        wh1[e] = np.concatenate([hv, ha], axis=1)
        bh1[e, 0] = np.concatenate([bv1 - bv1.mean(), ba1[e] - ba1[e].mean()])
        wh2[e, 0:64, 0] = Wv2[:, 0]
        wh2[e, 64:128, 1:33] = Wa2[e] - Wa2[e].mean(axis=1, keepdims=True)
        ba2t[e] = (ba2[e] - ba2[e].mean() + bv2[0])[None, :]

    g1 = np.asarray(inp["g1"], f8); g2 = np.asarray(inp["g2"], f8)
    g3 = np.asarray(inp["g3"], f8)
    gv = np.asarray(inp["gv"], f8); ga = np.asarray(inp["ga"], f8)
    g1t = g1.reshape(2, 128).T.astype(np.float32).copy()
    g2t = g2.reshape(2, 128).T.astype(np.float32).copy()
    g3t = g3.reshape(128, 1).astype(np.float32).copy()
    ght = np.stack(
        [np.concatenate([gv, ga[e]]).reshape(128, 1) for e in range(E)]
    ).astype(np.float32)

    # --- kappa calibration: per-layer constant rescale so the running
    # deferred scale stays O(1); lets the device drop the eps*s^2 chain.
    state = np.asarray(inp["state"], np.float64)
    tds = np.asarray(inp["time_delta"], np.float64)
    n = min(8192, state.shape[0])
    x = np.concatenate([state[:n], tds[:n, None], np.ones((n, 1))], axis=1).T  # [201,n]

    z1 = W1c.astype(np.float64).T @ x
    s1 = np.sqrt((z1**2).mean(axis=0) + 1e-5)
    k1 = float(1.0 / s1.mean())
    z1 *= k1; s1 *= k1
    u1 = np.maximum(z1, 0)
    z2 = W2c.astype(np.float64).T @ u1 + np.outer(b2c, s1)
    s2 = np.sqrt((z2**2).mean(axis=0) + 1e-5)
    k2 = float(1.0 / s2.mean())
    z2 *= k2; s2 *= k2
    u2 = np.maximum(z2, 0)
    z3 = W3c.astype(np.float64).T @ u2 + np.outer(b3c, s2)
    s3 = np.sqrt((z3**2).mean(axis=0) + 1e-5)
    k3 = float(1.0 / s3.mean())
    z3 *= k3; s3 *= k3
    u3 = np.maximum(z3, 0)
    hs = []
    for e in range(E):
        h = wh1[e].astype(np.float64).T @ u3 + np.outer(bh1[e, 0], s3)
        hs.append(np.sqrt((h[0:64]**2).mean(axis=0) + 1e-5))
        hs.append(np.sqrt((h[64:128]**2).mean(axis=0) + 1e-5))
    kh = float(1.0 / np.concatenate(hs).mean())

    W1c = (W1c * k1).astype(np.float32)
    W2c = (W2c * k2).astype(np.float32); b2c = (b2c * k2).astype(np.float32)
    W3c = (W3c * k3).astype(np.float32); b3c = (b3c * k3).astype(np.float32)
    wh1 = (wh1 * kh).astype(np.float32); bh1 = (bh1 * kh).astype(np.float32)

    ones256 = np.zeros((128, 2), np.float32); ones256[:, 0] = 1.0 / 256
    ones128 = np.zeros((128, 2), np.float32); ones128[:, 0] = 1.0 / 128
    gones = np.zeros((128, 33), np.float32)
    gones[0:64, 0] = 1.0 / 64
    gones[64:128, 32] = 1.0 / 64

    return {
        "w1k0": W1c[0:128].copy(), "w1k1": W1c[128:201].copy(),
        "w2k0": W2c[0:128].copy(), "w2k1": W2c[128:256].copy(),
        "w3k0": W3c[0:128].copy(), "w3k1": W3c[128:256].copy(),
        "b2row": b2c[None, :].copy(), "b3row": b3c[None, :].copy(),
        "wh1": wh1, "bh1": bh1, "wh2": wh2, "ba2t": ba2t,
        "g1t": g1t, "g2t": g2t, "g3t": g3t, "ght": ght,
        "ones256": ones256, "ones128": ones128, "gones": gones,
        "epsc": np.full((128, 1), EPS, np.float32),
    }


def _make_runner(nc):
    """Replicate bass2jax.run_bass_via_pjrt's multi-core path without output
    donation, returning a reusable jitted callable for repeat-timing."""
    import jax
    import numpy as np
    from jax.experimental.shard_map import shard_map
    from jax.sharding import Mesh, NamedSharding, PartitionSpec
    from concourse import mybir
    from concourse.bass2jax import (
        _bass_exec_p, install_neuronx_cc_hook, partition_id_tensor,
    )

    install_neuronx_cc_hook()
    partition_name = (nc.partition_id_tensor.name
                      if nc.partition_id_tensor else None)
    in_names, out_names, out_avals, zero_outs = [], [], [], []
    for alloc in nc.m.functions[0].allocations:
        if not isinstance(alloc, mybir.MemoryLocationSet):
            continue
        name = alloc.memorylocations[0].name
        if alloc.kind == "ExternalInput":
            if name != partition_name:
                in_names.append(name)
        elif alloc.kind == "ExternalOutput":
            out_names.append(name)
            shape = tuple(alloc.tensor_shape)
            dtype = mybir.dt.np(alloc.dtype)
            out_avals.append(jax.core.ShapedArray(shape, dtype))
            zero_outs.append(np.zeros(shape, dtype))
    n_params = len(in_names)
    all_in = in_names + out_names
    if partition_name is not None:
        all_in.append(partition_name)

    def _body(*args):
        operands = list(args)
        if partition_name is not None:
            operands.append(partition_id_tensor())
        return tuple(_bass_exec_p.bind(
            *operands,
            out_avals=tuple(out_avals), in_names=tuple(all_in),
            out_names=tuple(out_names), lowering_input_output_aliases=(),
            sim_require_finite=True, sim_require_nnan=True, nc=nc,
        ))

    devices = jax.devices()[:NCORES]
    mesh = Mesh(np.asarray(devices), ("core",))
    spec = PartitionSpec("core")
    fn = jax.jit(shard_map(
        _body, mesh=mesh, in_specs=(spec,) * (n_params + len(out_names)),
        out_specs=(spec,) * len(out_names), check_rep=False,
    ), keep_unused=True)
    sharding = NamedSharding(mesh, spec)
    return fn, in_names, zero_outs, sharding


def time_kernel(inputs, iters=16):
    """Estimate per-execution device time by dispatch-pipelined wall clock."""
    import time as _time
    import jax
    prep = _prepare(inputs)
    nc = prep["nc"]
    fn, in_names, zero_outs, sharding = _make_runner(nc)
    cat = [np.concatenate([m[n] for m in prep["in_maps"]], axis=0)
           for n in in_names]
    cat += [np.zeros((NCORES * z.shape[0], *z.shape[1:]), z.dtype)
            for z in zero_outs]
    dev = [jax.device_put(a, sharding) for a in cat]
    out = fn(*dev); jax.block_until_ready(out)   # compile+warm
    out = fn(*dev); jax.block_until_ready(out)

    def run_n(n):
        t0 = _time.perf_counter()
        o = None
        for _ in range(n):
            o = fn(*dev)
        jax.block_until_ready(o)
        return _time.perf_counter() - t0

    t1 = min(run_n(1) for _ in range(3))
    tn = min(run_n(1 + iters) for _ in range(3))
    per_iter = (tn - t1) / iters
    return per_iter, t1


def _prepare(inputs):
    state = np.asarray(inputs["state"], np.float32)
    td = np.asarray(inputs["time_delta"], np.float32)
    ev = np.asarray(inputs["event_type"]).astype(np.int64)
    B = state.shape[0]

    order = np.argsort(ev, kind="stable")
    ev_sorted = ev[order]
    groups = [order[ev_sorted == e] for e in range(E)]
    parts = [np.array_split(groups[e], NCORES) for e in range(E)]
    P_e = []
    for e in range(E):
        mx = max(len(parts[e][c]) for c in range(NCORES))
        P_e.append(int(np.ceil(mx / TILE)) * TILE if mx else 0)
    R = sum(P_e)
    tile_events = []
    for e in range(E):
        tile_events += [e] * (P_e[e] // TILE)

    seg0 = np.cumsum([0] + P_e[:-1])
    rowmap = np.full((NCORES, R), -1, np.int64)
    for e in range(E):
        for c in range(NCORES):
            p = parts[e][c]
            rowmap[c, seg0[e]:seg0[e] + len(p)] = p
    valid = rowmap >= 0

    xT = np.zeros((NCORES, D_IN + 1, R), np.float32)
    for c in range(NCORES):
        rc = rowmap[c]
        v = valid[c]
        xT[c, 0:S_DIM, v] = state[rc[v]]  # advanced indexing puts mask axis first
        xT[c, S_DIM, v] = td[rc[v]]
        xT[c, S_DIM + 1, v] = 1.0

    wts = _prep_weights(inputs)
    key = (R, tuple(tile_events))
    if key not in _PROG_CACHE:
        _PROG_CACHE[key] = _build_program(R, tile_events)
    return {
        "nc": _PROG_CACHE[key], "B": B, "rowmap": rowmap, "valid": valid,
        "in_maps": [dict(wts, xT=xT[c]) for c in range(NCORES)],
    }


def kernel(**inputs):
    global LAST_EXEC_NS
    from concourse.bass_utils import run_bass_kernel_spmd

    prep = _prepare(inputs)
    trace = bool(int(os.environ.get("KTRACE", "0")))
    tkw = {}
    if trace and os.environ.get("KTRACE_DIR"):
        os.makedirs(os.environ["KTRACE_DIR"], exist_ok=True)
        tkw["tmpdir"] = os.environ["KTRACE_DIR"]
    res = run_bass_kernel_spmd(
        prep["nc"], prep["in_maps"], core_ids=list(range(NCORES)), trace=trace,
        **tkw,
    )
    LAST_EXEC_NS = res.exec_time_ns

    out = np.empty((prep["B"], A), np.float32)
    rowmap, valid = prep["rowmap"], prep["valid"]
    for c in range(NCORES):
        v = valid[c]
        out[rowmap[c][v]] = res.results[c]["out"][v]
    return out

